# revision 27
# baseline (speedup 1.0000x reference)
"""Deformable self-attention TRN2 kernel.

Problem (hardcoded shapes): B=2, Lq=4096, S=16384, D=256, H=8, P=4 (32 slots/query).

Sharding: 8 cores; core k handles batch k//4, query rows [(k%4)*1024, (k%4+1)*1024).

v2: linear interpolation is rewritten as  a*V[r] + b*(V[r+1]-V[r])  with a=w,
b=w*w1, so each slot needs one 768-byte packed HBM record
[V[r]: 256 f16 | D[r]=V[r+1]-V[r]: 256 int8 (global scale)] instead of the
1KB f16 row-pair — 25% less gather traffic (the DMA-bus roofline term).
The int8 delta's global scale is folded into the per-slot b weights on
device (sdcol input). D decodes int8->f16 on DVE, overlapped with gather.

Per-core pipeline (8 tiles of 128 queries):
  A) PE: pos/attn projections (qT chunks as lhsT); DVE/ACT: softmax, floor
     (magic-constant), clip, edge masks -> per-record combine weights a/b;
     PE transposes pack them into block-diagonal lhs matrices and the
     clipped indices into the 16-partition-wrapped int16 stream dma_gather
     expects (replicated to all 8 Q7 core groups via SBUF->SBUF DMA).
  B) dma_gather (4096 idxs x 768B records), DVE casts the D region to f16,
     then the combine runs on PE with gathered tiles as the stationary
     operand: attnT[d, l] += V[:, j].T @ A[:, 4j:4j+4] + D[:, j].T @ B[...];
     this directly yields the transposed activation needed for the W_out
     projection (fused bias via rank-1 matmul).
"""

import sys

sys.path.insert(0, "/opt/trn_rl_repo")

import numpy as np

import concourse.bass as bass
import concourse.mybir as mybir
import concourse.tile as tile
from concourse import bacc
from concourse.masks import make_identity

F32 = mybir.dt.float32
F16 = mybir.dt.float16
I16 = mybir.dt.int16
I8 = mybir.dt.int8
AX = mybir.AxisListType
OP = mybir.AluOpType
ACTF = mybir.ActivationFunctionType

B, LQ, S, D = 2, 4096, 16384, 256
NHEAD, NPOINT = 8, 4
NSLOT = NHEAD * NPOINT          # 32 sampling slots per query
N_CORES = 8
LQ_SHARD = LQ * B // N_CORES    # 1024 queries per core
QT = 128                        # queries per tile
NT = LQ_SHARD // QT             # 8 tiles
NJ = QT * NSLOT // 128          # 32 gather columns per tile
REC = 768                       # packed record bytes: 512 (V f16) + 256 (D int8)
CPOS = 0.5 * (S - 1)            # grid_sample coord scale
MAGIC = 12582912.0              # 1.5 * 2^23 fp32 round-to-int magic

_CACHE = {}


def build_program(gather_queues=4, g_bufs=4, gd_bufs=3, repeat=1,
                  calls_per_tile=2, at_bufs=2, out_bufs=1, decode_eng="mix"):
    nc = bacc.Bacc("TRN2", target_bir_lowering=False, debug=False,
                   num_swdge_queues=gather_queues)

    def din(name, shape):
        return nc.dram_tensor(name, list(shape), F32, kind="ExternalInput").ap()

    qT = din("qT", [128, 2, LQ_SHARD])
    value = nc.dram_tensor("value", [S, REC], I8, kind="ExternalInput").ap()
    # packed constants: one wide DMA each instead of 15 serialized HWDGE gens
    # cpack cols: wpos[2x32] | wattn[2x32] | wout[2x256] | sd[1]
    cpack = din("cpack", [128, 4 * NSLOT + 2 * D + 1])
    # rpack cols: bpos[32] | battn[32] | bout[256] | refy[1024]
    rpack = din("rpack", [1, 2 * NSLOT + D + LQ_SHARD])
    out_d = nc.dram_tensor("out", [LQ_SHARD, D], F32, kind="ExternalOutput").ap()

    with tile.TileContext(nc) as tc:
        with tc.tile_pool(name="consts", bufs=1) as cpool, \
             tc.tile_pool(name="work", bufs=2) as wpool, \
             tc.tile_pool(name="gbuf", bufs=g_bufs) as gpool, \
             tc.tile_pool(name="gdbuf", bufs=gd_bufs) as gdpool, \
             tc.tile_pool(name="ps_proj", bufs=1, space="PSUM") as ps_proj, \
             tc.tile_pool(name="ps_tr", bufs=1, space="PSUM") as ps_tr, \
             tc.tile_pool(name="ps_at", bufs=at_bufs, space="PSUM") as ps_at, \
             tc.tile_pool(name="ps_out", bufs=out_bufs, space="PSUM") as ps_out:

            # ---- constants / persistent tiles ----
            cp_sb = cpool.tile([128, 4 * NSLOT + 2 * D + 1], F32)
            nc.sync.dma_start(out=cp_sb[:], in_=cpack[:])
            rp_sb = cpool.tile([1, 2 * NSLOT + D + LQ_SHARD], F32)
            nc.sync.dma_start(out=rp_sb[:], in_=rpack[:])

            def wpos_sb(c):
                return cp_sb[:, c * NSLOT:(c + 1) * NSLOT]

            def wattn_sb(c):
                return cp_sb[:, (2 + c) * NSLOT:(3 + c) * NSLOT]

            def wout_sb(c):
                return cp_sb[:, 4 * NSLOT + c * D:4 * NSLOT + (c + 1) * D]

            sd_sb = cp_sb[:, 4 * NSLOT + 2 * D:4 * NSLOT + 2 * D + 1]
            bpos_sb = rp_sb[:1, 0:NSLOT]
            battn_sb = rp_sb[:1, NSLOT:2 * NSLOT]
            bout_sb = rp_sb[:1, 2 * NSLOT:2 * NSLOT + D]
            refy_sb = rp_sb[:1, 2 * NSLOT + D:]

            # qT loaded tile-by-tile so tile 0's phase A starts early
            qT_sb = cpool.tile([128, 2, LQ_SHARD], F32)
            for t in range(NT):
                tsl = bass.ds(t * QT, QT)
                nc.sync.dma_start(out=qT_sb[:, :, tsl], in_=qT[:, :, tsl])

            refpos = cpool.tile([1, LQ_SHARD], F32)
            # refpos = (refy + 1) * CPOS = refy*CPOS + CPOS
            nc.scalar.activation(refpos[:], refy_sb, ACTF.Copy, bias=float(CPOS), scale=float(CPOS))

            ident = cpool.tile([128, 128], F32)
            make_identity(nc, ident[:])
            ones_row = cpool.tile([1, 128], F32)
            nc.vector.memset(ones_row[:], 1.0)
            ones32 = cpool.tile([1, NSLOT], F32)
            nc.vector.memset(ones32[:], 1.0)
            c0 = cpool.tile([128, 1], F32)
            nc.vector.memset(c0[:], 0.0)
            cbig = cpool.tile([128, 1], F32)
            nc.vector.memset(cbig[:], MAGIC)
            cm1 = cpool.tile([128, 1], F32)
            nc.vector.memset(cm1[:], -1.0)
            chi = cpool.tile([128, 1], F32)      # S-1: max record index / OOB-high mask
            nc.vector.memset(chi[:], float(S - 1))
            cmid = cpool.tile([128, 1], F32)     # S-2: last interior start row
            nc.vector.memset(cmid[:], float(S - 2))

            def bc(t):
                return t[:, :1].to_broadcast([128, NSLOT])

            # f16 copies of the output projection weights (f32 PE matmuls run
            # at quarter rate; f16 rounding is negligible at this tolerance)
            woutf = cpool.tile([128, 2, D], F16)
            nc.vector.tensor_copy(out=woutf[:, 0, :], in_=wout_sb(0))
            nc.vector.tensor_copy(out=woutf[:, 1, :], in_=wout_sb(1))
            boutf = cpool.tile([1, D], F16)
            nc.vector.tensor_copy(out=boutf[:], in_=bout_sb)
            onesf = cpool.tile([1, 128], F16)
            nc.vector.memset(onesf[:], 1.0)

            # per-tile persistent phase-A outputs
            lhs0s = [cpool.tile([128, 128], F16, name=f"lhs0_t{t}") for t in range(NT)]
            lhs1s = [cpool.tile([128, 128], F16, name=f"lhs1_t{t}") for t in range(NT)]
            idxts = [cpool.tile([128, 2 * QT], I16, name=f"idxt_t{t}") for t in range(NT)]
            for t in range(NT):
                nc.vector.memset(lhs0s[t][:], 0.0)
                nc.vector.memset(lhs1s[t][:], 0.0)

            for rep in range(repeat):
                # ---------- loop 1: phase A for all tiles ----------
                for t in range(NT):
                    lsl = bass.ds(t * QT, QT)
                    projps = ps_proj.tile([128, 2 * NSLOT], F32)
                    logits_ps = projps[:, 0:NSLOT]
                    pos_ps = projps[:, NSLOT:2 * NSLOT]
                    # pos projection first: the idx chain (critical path to
                    # the gather) depends only on it
                    nc.tensor.matmul(pos_ps, lhsT=qT_sb[:, 0, lsl], rhs=wpos_sb(0), start=True, stop=False)
                    nc.tensor.matmul(pos_ps, lhsT=qT_sb[:, 1, lsl], rhs=wpos_sb(1), start=False, stop=False)
                    nc.tensor.matmul(pos_ps, lhsT=refpos[:1, lsl], rhs=ones32[:1, :], start=False, stop=False)
                    nc.tensor.matmul(pos_ps, lhsT=ones_row[:1, :], rhs=bpos_sb, start=False, stop=True)
                    nc.tensor.matmul(logits_ps, lhsT=qT_sb[:, 0, lsl], rhs=wattn_sb(0), start=True, stop=False)
                    nc.tensor.matmul(logits_ps, lhsT=qT_sb[:, 1, lsl], rhs=wattn_sb(1), start=False, stop=False)
                    nc.tensor.matmul(logits_ps, lhsT=ones_row[:1, :], rhs=battn_sb, start=False, stop=True)

                    # ---- idx chain (feeds the gather; keep it short) ----
                    pos = wpool.tile([128, NSLOT], F32)
                    nc.vector.tensor_copy(out=pos[:], in_=pos_ps)
                    rnd = wpool.tile([128, NSLOT], F32)
                    nc.vector.tensor_tensor(out=rnd[:], in0=pos[:], in1=bc(cbig), op=OP.add)
                    nc.vector.tensor_tensor(out=rnd[:], in0=rnd[:], in1=bc(cbig), op=OP.subtract)
                    gt = wpool.tile([128, NSLOT], F32)
                    nc.vector.tensor_tensor(out=gt[:], in0=rnd[:], in1=pos[:], op=OP.is_gt)
                    i0 = wpool.tile([128, NSLOT], F32)
                    nc.vector.tensor_tensor(out=i0[:], in0=rnd[:], in1=gt[:], op=OP.subtract)
                    # r = clip(i0, 0, S-1): record index (self-contained, so S-1 ok)
                    r = wpool.tile([128, NSLOT], F32)
                    nc.vector.tensor_tensor(out=r[:], in0=i0[:], in1=bc(chi), op=OP.min)
                    nc.vector.tensor_tensor(out=r[:], in0=r[:], in1=bc(c0), op=OP.max)
                    # replicate idx columns 8x in the free dim, then one
                    # transpose per 16-slot half directly yields all 128
                    # partitions (16-wrap x 8 Q7 groups) -- no SBUF->SBUF DMAs
                    rrep = wpool.tile([128, 2, 128], F32)
                    nc.vector.tensor_copy(out=rrep[:, 0, 0:16], in_=r[:, 0:16])
                    nc.vector.tensor_copy(out=rrep[:, 1, 0:16], in_=r[:, 16:32])
                    nc.vector.tensor_copy(out=rrep[:, :, 16:32], in_=rrep[:, :, 0:16])
                    nc.vector.tensor_copy(out=rrep[:, :, 32:64], in_=rrep[:, :, 0:32])
                    nc.vector.tensor_copy(out=rrep[:, :, 64:128], in_=rrep[:, :, 0:64])
                    trr = ps_tr.tile([128, 2, 128], F32, name="trr")
                    nc.tensor.transpose(trr[:, 0, :], in_=rrep[:, 0, :], identity=ident[:])
                    nc.tensor.transpose(trr[:, 1, :], in_=rrep[:, 1, :], identity=ident[:])
                    idxt = idxts[t]
                    nc.vector.tensor_copy(out=idxt[:, 0:2 * QT:2], in_=trr[:, 0, :])
                    nc.vector.tensor_copy(out=idxt[:, 1:2 * QT:2], in_=trr[:, 1, :])

                    # ---- combine weights (only needed once the data lands) ----
                    w1 = wpool.tile([128, NSLOT], F32)
                    nc.vector.tensor_tensor(out=w1[:], in0=pos[:], in1=i0[:], op=OP.subtract)
                    negmax = wpool.tile([128, 1], F32)
                    nc.vector.tensor_reduce(negmax[:], logits_ps, AX.X, OP.max, negate=True)
                    w = wpool.tile([128, NSLOT], F32)
                    sums = wpool.tile([128, 1], F32)
                    nc.scalar.activation(w[:], logits_ps, ACTF.Exp, bias=negmax[:, :1], accum_out=sums[:, :1])
                    rec = wpool.tile([128, 1], F32)
                    nc.vector.reciprocal(rec[:], sums[:])
                    nc.vector.tensor_tensor(out=w[:], in0=w[:], in1=rec[:, :1].to_broadcast([128, NSLOT]), op=OP.mult)
                    # masks
                    mge = wpool.tile([128, NSLOT], F32)
                    nc.vector.tensor_tensor(out=mge[:], in0=i0[:], in1=bc(c0), op=OP.is_ge)
                    mle = wpool.tile([128, NSLOT], F32)
                    nc.vector.tensor_tensor(out=mle[:], in0=i0[:], in1=bc(cmid), op=OP.is_le)
                    mmid = wpool.tile([128, NSLOT], F32)
                    nc.vector.tensor_tensor(out=mmid[:], in0=mge[:], in1=mle[:], op=OP.mult)
                    mlo = wpool.tile([128, NSLOT], F32)
                    nc.vector.tensor_tensor(out=mlo[:], in0=i0[:], in1=bc(cm1), op=OP.is_equal)
                    mhi = wpool.tile([128, NSLOT], F32)
                    nc.vector.tensor_tensor(out=mhi[:], in0=i0[:], in1=bc(chi), op=OP.is_equal)
                    onem = wpool.tile([128, NSLOT], F32)
                    nc.scalar.activation(onem[:], w1[:], ACTF.Copy, bias=1.0, scale=-1.0)
                    # a = w * (mmid + w1*mlo + (1-w1)*mhi)
                    ta = wpool.tile([128, NSLOT], F32)
                    nc.vector.tensor_tensor(out=ta[:], in0=w1[:], in1=mlo[:], op=OP.mult)
                    tb = wpool.tile([128, NSLOT], F32)
                    nc.vector.tensor_tensor(out=tb[:], in0=onem[:], in1=mhi[:], op=OP.mult)
                    aw = wpool.tile([128, NSLOT], F32)
                    nc.vector.tensor_tensor(out=aw[:], in0=mmid[:], in1=ta[:], op=OP.add)
                    nc.vector.tensor_tensor(out=aw[:], in0=aw[:], in1=tb[:], op=OP.add)
                    nc.vector.tensor_tensor(out=aw[:], in0=aw[:], in1=w[:], op=OP.mult)
                    # b = w * w1 * mmid * sD
                    bw = wpool.tile([128, NSLOT], F32)
                    nc.vector.tensor_tensor(out=bw[:], in0=w1[:], in1=sd_sb[:, :1].to_broadcast([128, NSLOT]), op=OP.mult)
                    nc.vector.tensor_tensor(out=bw[:], in0=bw[:], in1=mmid[:], op=OP.mult)
                    nc.vector.tensor_tensor(out=bw[:], in0=bw[:], in1=w[:], op=OP.mult)

                    trw = ps_tr.tile([32, 2, 128], F32, name="trw")
                    nc.tensor.transpose(trw[0:32, 0, :], in_=aw[:], identity=ident[:])
                    nc.tensor.transpose(trw[0:32, 1, :], in_=bw[:], identity=ident[:])
                    lhs0, lhs1 = lhs0s[t], lhs1s[t]
                    for i in range(4):
                        nc.vector.tensor_copy(out=lhs0[32 * i:32 * i + 32, i:128:4], in_=trw[0:32, 0, i:128:4])
                        nc.vector.tensor_copy(out=lhs1[32 * i:32 * i + 32, i:128:4], in_=trw[0:32, 1, i:128:4])

                # ---------- loop 2: gather + decode + combine + output,
                # software-pipelined so tile t+1's gather/decode issue before
                # tile t's combine (keeps the in-order DVE/ACT queues from
                # stalling the next tile's decode behind output copies) ----------
                jpc = NJ // calls_per_tile            # j-columns per call
                npc = QT * NSLOT // calls_per_tile    # idxs per call
                Gs, Gds = {}, {}

                def gather_decode(t):
                    idxt = idxts[t]
                    G = gpool.tile([128, NJ, REC], I8, name="G")
                    Gd = gdpool.tile([128, NJ, D], F16, name="Gd")
                    Gs[t], Gds[t] = G, Gd
                    for gch in range(calls_per_tile):
                        jsl = bass.ds(jpc * gch, jpc)
                        nc.gpsimd.dma_gather(
                            out_ap=G[:, jsl, :],
                            in_ap=value,
                            idxs_ap=idxt[:, (npc // 16) * gch:(npc // 16) * (gch + 1)],
                            num_idxs=npc,
                            num_idxs_reg=npc,
                            elem_size=REC,
                            elem_step=REC,
                            single_packet=False,
                            queue_num=(t * calls_per_tile + gch) % gather_queues,
                        )
                        # decode the int8 delta region to f16, halves split
                        # DVE:ACT 2:2 to balance engine load
                        h = jpc // 2
                        for half in range(2):
                            hsl = bass.ds(jpc * gch + h * half, h)
                            hidx = gch * 2 + half
                            eng = "dve" if (decode_eng == "dve" or hidx in (0, 2)) else "act"
                            if eng == "act":
                                nc.scalar.activation(Gd[:, hsl, :], G[:, hsl, 512:768], ACTF.Copy)
                            else:
                                nc.vector.tensor_copy(out=Gd[:, hsl, :], in_=G[:, hsl, 512:768])

                def combine_out(t):
                    lhs0, lhs1 = lhs0s[t], lhs1s[t]
                    G, Gd = Gs.pop(t), Gds.pop(t)
                    at0 = ps_at.tile([128, 128], F32, name="at0")
                    at1 = ps_at.tile([128, 128], F32, name="at1")
                    atp = (at0, at1)
                    # all tap0 (V) matmuls first: they only need the gather,
                    # so PE proceeds while the int8 delta decode runs. One
                    # accumulation group per psum bank: the j=0 start marks
                    # the whole 2KB zero region, later j blocks lazily zero.
                    for j in range(NJ):
                        Gv = G[:, j, 0:512].bitcast(F16)      # [128, 256] f16 V row
                        for c in range(2):
                            nc.tensor.matmul(atp[c][:, 4 * j:4 * j + 4],
                                             lhsT=Gv[:, 128 * c:128 * c + 128],
                                             rhs=lhs0[:, 4 * j:4 * j + 4],
                                             start=(j == 0), stop=False)
                    for j in range(NJ):
                        for c in range(2):
                            nc.tensor.matmul(atp[c][:, 4 * j:4 * j + 4],
                                             lhsT=Gd[:, j, 128 * c:128 * c + 128],
                                             rhs=lhs1[:, 4 * j:4 * j + 4],
                                             start=False, stop=(j == NJ - 1))
                    attnT = wpool.tile([128, 2, 128], F16)
                    nc.vector.tensor_copy(out=attnT[:, 0, :], in_=at0[:])
                    nc.vector.tensor_copy(out=attnT[:, 1, :], in_=at1[:])

                    op_ = ps_out.tile([128, D], F32)
                    nc.tensor.matmul(op_[:], lhsT=attnT[:, 0, :], rhs=woutf[:, 0, :], start=True, stop=False)
                    nc.tensor.matmul(op_[:], lhsT=attnT[:, 1, :], rhs=woutf[:, 1, :], start=False, stop=False)
                    nc.tensor.matmul(op_[:], lhsT=onesf[:1, :], rhs=boutf[:1, :], start=False, stop=True)
                    out_sb = wpool.tile([128, D], F32)
                    nc.vector.tensor_copy(out=out_sb[:], in_=op_[:])
                    nc.sync.dma_start(out=out_d[bass.ds(t * QT, QT), :], in_=out_sb[:])

                for t in range(NT):
                    gather_decode(t)
                    if t >= 1:
                        combine_out(t - 1)
                combine_out(NT - 1)

    nc.compile()
    return nc


def pack_value(vb):
    """value[b] [S, 256] f32 -> packed [S, 768] int8 records + delta scale."""
    vf16 = vb.astype(np.float16)                                  # [S, 256]
    d = (vf16[1:, :].astype(np.float32) - vf16[:-1, :].astype(np.float32))
    sd = float(np.abs(d).max()) / 127.0 if d.size else 1.0
    dq = np.zeros((S, D), dtype=np.int8)
    dq[:-1, :] = np.clip(np.round(d / sd), -127, 127).astype(np.int8)
    packed = np.empty((S, REC), dtype=np.int8)
    packed[:, 0:512] = vf16.view(np.int8).reshape(S, 512)
    packed[:, 512:768] = dq
    return packed, sd


def make_in_maps(query, key, value, reference_points, W_off, b_off, W_attn, b_attn, W_out, b_out):
    query = np.asarray(query, dtype=np.float32)
    value = np.asarray(value, dtype=np.float32)
    reference_points = np.asarray(reference_points, dtype=np.float32)
    W_off = np.asarray(W_off, dtype=np.float32)
    b_off = np.asarray(b_off, dtype=np.float32)
    W_attn = np.asarray(W_attn, dtype=np.float32)
    b_attn = np.asarray(b_attn, dtype=np.float32)
    W_out = np.asarray(W_out, dtype=np.float32)
    b_out = np.asarray(b_out, dtype=np.float32)

    # fold grid_sample coordinate transform into the offset head (y columns only)
    wposf = (W_off[:, 1::2] * CPOS).astype(np.float32)           # [256, 32]
    bposf = (b_off[1::2] * CPOS).astype(np.float32)              # [32]

    def chunked(m, ncols):                                        # [256, n] -> [128, 2, n]
        return np.ascontiguousarray(m.reshape(2, 128, ncols).transpose(1, 0, 2))

    wpos_r = chunked(wposf, NSLOT)
    wattn_r = chunked(W_attn, NSLOT)
    wout_r = chunked(W_out, D)

    packs = [pack_value(value[b]) for b in range(B)]

    in_maps = []
    per_core = LQ // (N_CORES // B)                               # 1024
    for k in range(N_CORES):
        b = k // (N_CORES // B)
        q0 = (k % (N_CORES // B)) * per_core
        qs = query[b, q0:q0 + per_core, :]                        # [1024, 256]
        qT_r = np.ascontiguousarray(qs.T.reshape(2, 128, per_core).transpose(1, 0, 2))
        packed, sd = packs[b]
        cpack = np.empty((128, 4 * NSLOT + 2 * D + 1), dtype=np.float32)
        cpack[:, 0:NSLOT] = wpos_r[:, 0, :]
        cpack[:, NSLOT:2 * NSLOT] = wpos_r[:, 1, :]
        cpack[:, 2 * NSLOT:3 * NSLOT] = wattn_r[:, 0, :]
        cpack[:, 3 * NSLOT:4 * NSLOT] = wattn_r[:, 1, :]
        cpack[:, 4 * NSLOT:4 * NSLOT + D] = wout_r[:, 0, :]
        cpack[:, 4 * NSLOT + D:4 * NSLOT + 2 * D] = wout_r[:, 1, :]
        cpack[:, 4 * NSLOT + 2 * D] = sd
        rpack = np.empty((1, 2 * NSLOT + D + LQ_SHARD), dtype=np.float32)
        rpack[0, 0:NSLOT] = bposf
        rpack[0, NSLOT:2 * NSLOT] = b_attn.astype(np.float32)
        rpack[0, 2 * NSLOT:2 * NSLOT + D] = b_out.astype(np.float32)
        rpack[0, 2 * NSLOT + D:] = reference_points[b, q0:q0 + per_core, 1]
        in_maps.append({
            "qT": qT_r,
            "value": packed,
            "cpack": cpack,
            "rpack": rpack,
        })
    return in_maps


def kernel(**inputs) -> np.ndarray:
    from concourse.bass_utils import run_bass_kernel_spmd

    if "nc" not in _CACHE:
        _CACHE["nc"] = build_program()
    nc = _CACHE["nc"]
    in_maps = make_in_maps(**inputs)
    res = run_bass_kernel_spmd(nc, in_maps, list(range(N_CORES)), trace=False)
    shards = [res.results[k]["out"] for k in range(N_CORES)]
    out = np.empty((B, LQ, D), dtype=np.float32)
    per_core = LQ // (N_CORES // B)
    for k in range(N_CORES):
        b = k // (N_CORES // B)
        q0 = (k % (N_CORES // B)) * per_core
        out[b, q0:q0 + per_core, :] = shards[k]
    return out


# revision 39
# speedup vs baseline: 2.3953x; 2.3953x over previous
"""Deformable self-attention TRN2 kernel.

Problem (hardcoded shapes): B=2, Lq=4096, S=16384, D=256, H=8, P=4 (32 slots/query).

Sharding: 8 cores; core k handles batch k//4, query rows [(k%4)*1024, (k%4+1)*1024).
Each core holds the full value[b] sequence in its HBM and gathers sampled rows
from it with the SWDGE dma_gather instruction (one 1KB f16 row-pair per slot),
spread across all 4 SWDGE queues with single_packet=False (each ~4.5x faster
than the single-queue single-packet default; measured 286us -> 63us/call).

Per-core pipeline (8 tiles of 128 queries):
  A) PE: pos/attn projections (qT chunks as lhsT); DVE/ACT: softmax, floor
     (magic-constant), clip, edge masks -> per-tap combine weights wt0/wt1;
     PE transposes pack weights into block-diagonal lhs matrices and the
     clipped indices into the 16-partition-wrapped int16 stream dma_gather
     expects (replicated to all 8 Q7 core groups via SBUF->SBUF DMA).
  B) dma_gather (4096 idxs x 2KB: value rows [r, r+1] per slot), then the
     combine runs on PE with the gathered tile as the stationary operand:
     attnT[d, l] += G[:, j, tap-half].T @ W{tap}[:, 4j:4j+4]; this directly
     yields the transposed activation needed for the W_out projection
     (fused bias via rank-1 matmul), so no intermediate transposes/copies.
"""

import sys

sys.path.insert(0, "/opt/trn_rl_repo")

import numpy as np

import concourse.bass as bass
import concourse.mybir as mybir
import concourse.tile as tile
from concourse import bacc
from concourse.masks import make_identity

F32 = mybir.dt.float32
F16 = mybir.dt.float16
I16 = mybir.dt.int16
AX = mybir.AxisListType
OP = mybir.AluOpType
ACTF = mybir.ActivationFunctionType

B, LQ, S, D = 2, 4096, 16384, 256
NHEAD, NPOINT = 8, 4
NSLOT = NHEAD * NPOINT          # 32 sampling slots per query
N_CORES = 8
LQ_SHARD = LQ * B // N_CORES    # 1024 queries per core
QT = 128                        # queries per tile
NT = LQ_SHARD // QT             # 8 tiles
NJ = QT * NSLOT // 128          # 32 gather columns per tile
CPOS = 0.5 * (S - 1)            # grid_sample coord scale
MAGIC = 12582912.0              # 1.5 * 2^23 fp32 round-to-int magic

_CACHE = {}


def build_program(gather_queues=4, g_bufs=3, idx_bufs=2, repeat=1, combine=True,
                  hoist_idx=False, calls_per_tile=4, at_bufs=1, out_bufs=1,
                  combine_mode="std"):
    if not combine:
        combine_mode = "std"
    nc = bacc.Bacc("TRN2", target_bir_lowering=False, debug=False,
                   num_swdge_queues=gather_queues)

    def din(name, shape):
        return nc.dram_tensor(name, list(shape), F32, kind="ExternalInput").ap()

    qT = din("qT", [128, 2, LQ_SHARD])
    refy = din("refy", [1, LQ_SHARD])
    value = nc.dram_tensor("value", [S, D], F16, kind="ExternalInput").ap()
    wpos = din("wpos", [128, 2, NSLOT])
    bpos = din("bpos", [1, NSLOT])
    wattn = din("wattn", [128, 2, NSLOT])
    battn = din("battn", [1, NSLOT])
    wout = din("wout", [128, 2, D])
    bout = din("bout", [1, D])
    out_d = nc.dram_tensor("out", [LQ_SHARD, D], F32, kind="ExternalOutput").ap()

    # overlapping row-pair view of value: row r -> value[r:r+2, :] flattened (512 f32)
    val2 = bass.AP(value.tensor, 0, [[D, S - 1], [1, 2 * D]])

    with tile.TileContext(nc) as tc:
        with tc.tile_pool(name="consts", bufs=1) as cpool, \
             tc.tile_pool(name="work", bufs=2) as wpool, \
             tc.tile_pool(name="gbuf", bufs=g_bufs) as gpool, \
             tc.tile_pool(name="ps_proj", bufs=1, space="PSUM") as ps_proj, \
             tc.tile_pool(name="ps_tr", bufs=1, space="PSUM") as ps_tr, \
             tc.tile_pool(name="ps_at", bufs=at_bufs, space="PSUM") as ps_at, \
             tc.tile_pool(name="ps_out", bufs=out_bufs, space="PSUM") as ps_out:

            # ---- constants / persistent tiles ----
            qT_sb = cpool.tile([128, 2, LQ_SHARD], F32)
            nc.sync.dma_start(out=qT_sb[:], in_=qT[:])
            wpos_sb = cpool.tile([128, 2, NSLOT], F32)
            nc.sync.dma_start(out=wpos_sb[:], in_=wpos[:])
            wattn_sb = cpool.tile([128, 2, NSLOT], F32)
            nc.sync.dma_start(out=wattn_sb[:], in_=wattn[:])
            bpos_sb = cpool.tile([1, NSLOT], F32)
            nc.sync.dma_start(out=bpos_sb[:], in_=bpos[:])
            battn_sb = cpool.tile([1, NSLOT], F32)
            nc.sync.dma_start(out=battn_sb[:], in_=battn[:])
            wout_sb = cpool.tile([128, 2, D], F32)
            nc.sync.dma_start(out=wout_sb[:], in_=wout[:])
            bout_sb = cpool.tile([1, D], F32)
            nc.sync.dma_start(out=bout_sb[:], in_=bout[:])
            # f16 copies for the output projection: f32 PE matmuls run at
            # quarter rate; f16 rounding is negligible at this tolerance
            woutf = cpool.tile([128, 2, D], F16)
            nc.vector.tensor_copy(out=woutf[:], in_=wout_sb[:])
            boutf = cpool.tile([1, D], F16)
            nc.vector.tensor_copy(out=boutf[:], in_=bout_sb[:])
            onesf = cpool.tile([1, 128], F16)
            nc.vector.memset(onesf[:], 1.0)
            refy_sb = cpool.tile([1, LQ_SHARD], F32)
            nc.sync.dma_start(out=refy_sb[:], in_=refy[:])

            refpos = cpool.tile([1, LQ_SHARD], F32)
            # refpos = (refy + 1) * CPOS = refy*CPOS + CPOS
            nc.scalar.activation(refpos[:], refy_sb[:], ACTF.Copy, bias=float(CPOS), scale=float(CPOS))

            ident = cpool.tile([128, 128], F32)
            make_identity(nc, ident[:])
            identf16 = cpool.tile([128, 128], F16)
            nc.vector.tensor_copy(out=identf16[:], in_=ident[:])
            ones_row = cpool.tile([1, 128], F32)
            nc.vector.memset(ones_row[:], 1.0)
            ones32 = cpool.tile([1, NSLOT], F32)
            nc.vector.memset(ones32[:], 1.0)
            c0 = cpool.tile([128, 1], F32)
            nc.vector.memset(c0[:], 0.0)
            cone = cpool.tile([128, 1], F32)
            nc.vector.memset(cone[:], 1.0)
            cbig = cpool.tile([128, 1], F32)
            nc.vector.memset(cbig[:], MAGIC)
            cm1 = cpool.tile([128, 1], F32)
            nc.vector.memset(cm1[:], -1.0)
            chi = cpool.tile([128, 1], F32)      # S-1: first OOB-high start
            nc.vector.memset(chi[:], float(S - 1))
            cclip = cpool.tile([128, 1], F32)    # S-2: max gather start row
            nc.vector.memset(cclip[:], float(S - 2))

            def bc(t):
                return t[:, :1].to_broadcast([128, NSLOT])


            # per-tile persistent phase-A outputs (reused across repeat reps)
            lhs0s = [cpool.tile([128, 128], F16, name=f"lhs0_t{t}") for t in range(NT)]
            lhs1s = [cpool.tile([128, 128], F16, name=f"lhs1_t{t}") for t in range(NT)]
            # tap1 swap-path weight slabs: slab (b,u) at [:, b, 36u:36u+32] holds
            # the [128, 32] stationary for group b's queries, with only cols
            # 4u..4u+4 nonzero (zeros persist from the one-time memset).
            lhsw1s = ([cpool.tile([128, 4, 288], F16, name=f"lhsw1_t{t}") for t in range(NT)]
                      if combine_mode == "mixed" else [None] * NT)
            idxts = [cpool.tile([128, 2 * QT], I16, name=f"idxt_t{t}") for t in range(NT)]
            for t in range(NT):
                nc.vector.memset(lhs0s[t][:], 0.0)
                nc.vector.memset(lhs1s[t][:], 0.0)
                if combine_mode == "mixed":
                    nc.vector.memset(lhsw1s[t][:], 0.0)

            for rep in range(repeat):
                # ---------- loop 1: phase A for all tiles (keeps the PE queue
                # ahead of the gather stream so the SWDGE queue never starves) ----------
                for t in range(NT) if (rep == 0 or not hoist_idx) else []:
                    lsl = bass.ds(t * QT, QT)
                    projps = ps_proj.tile([128, 2 * NSLOT], F32)
                    logits_ps = projps[:, 0:NSLOT]
                    pos_ps = projps[:, NSLOT:2 * NSLOT]
                    nc.tensor.matmul(logits_ps, lhsT=qT_sb[:, 0, lsl], rhs=wattn_sb[:, 0, :], start=True, stop=False)
                    nc.tensor.matmul(logits_ps, lhsT=qT_sb[:, 1, lsl], rhs=wattn_sb[:, 1, :], start=False, stop=False)
                    nc.tensor.matmul(logits_ps, lhsT=ones_row[:1, :], rhs=battn_sb[:1, :], start=False, stop=True)
                    nc.tensor.matmul(pos_ps, lhsT=qT_sb[:, 0, lsl], rhs=wpos_sb[:, 0, :], start=True, stop=False)
                    nc.tensor.matmul(pos_ps, lhsT=qT_sb[:, 1, lsl], rhs=wpos_sb[:, 1, :], start=False, stop=False)
                    nc.tensor.matmul(pos_ps, lhsT=refpos[:1, lsl], rhs=ones32[:1, :], start=False, stop=False)
                    nc.tensor.matmul(pos_ps, lhsT=ones_row[:1, :], rhs=bpos_sb[:1, :], start=False, stop=True)

                    negmax = wpool.tile([128, 1], F32)
                    nc.vector.tensor_reduce(negmax[:], logits_ps, AX.X, OP.max, negate=True)
                    w = wpool.tile([128, NSLOT], F32)
                    sums = wpool.tile([128, 1], F32)
                    nc.scalar.activation(w[:], logits_ps, ACTF.Exp, bias=negmax[:, :1], accum_out=sums[:, :1])
                    rec = wpool.tile([128, 1], F32)
                    nc.vector.reciprocal(rec[:], sums[:])
                    nc.vector.tensor_tensor(out=w[:], in0=w[:], in1=rec[:, :1].to_broadcast([128, NSLOT]), op=OP.mult)

                    pos = wpool.tile([128, NSLOT], F32)
                    nc.vector.tensor_copy(out=pos[:], in_=pos_ps)
                    rnd = wpool.tile([128, NSLOT], F32)
                    nc.vector.tensor_tensor(out=rnd[:], in0=pos[:], in1=bc(cbig), op=OP.add)
                    nc.vector.tensor_tensor(out=rnd[:], in0=rnd[:], in1=bc(cbig), op=OP.subtract)
                    gt = wpool.tile([128, NSLOT], F32)
                    nc.vector.tensor_tensor(out=gt[:], in0=rnd[:], in1=pos[:], op=OP.is_gt)
                    i0 = wpool.tile([128, NSLOT], F32)
                    nc.vector.tensor_tensor(out=i0[:], in0=rnd[:], in1=gt[:], op=OP.subtract)
                    w1 = wpool.tile([128, NSLOT], F32)
                    nc.vector.tensor_tensor(out=w1[:], in0=pos[:], in1=i0[:], op=OP.subtract)
                    r = wpool.tile([128, NSLOT], F32)
                    nc.vector.tensor_tensor(out=r[:], in0=i0[:], in1=bc(cclip), op=OP.min)
                    nc.vector.tensor_tensor(out=r[:], in0=r[:], in1=bc(c0), op=OP.max)
                    mge = wpool.tile([128, NSLOT], F32)
                    nc.vector.tensor_tensor(out=mge[:], in0=i0[:], in1=bc(c0), op=OP.is_ge)
                    mle = wpool.tile([128, NSLOT], F32)
                    nc.vector.tensor_tensor(out=mle[:], in0=i0[:], in1=bc(cclip), op=OP.is_le)
                    mmid = wpool.tile([128, NSLOT], F32)
                    nc.vector.tensor_tensor(out=mmid[:], in0=mge[:], in1=mle[:], op=OP.mult)
                    mlo = wpool.tile([128, NSLOT], F32)
                    nc.vector.tensor_tensor(out=mlo[:], in0=i0[:], in1=bc(cm1), op=OP.is_equal)
                    mhi = wpool.tile([128, NSLOT], F32)
                    nc.vector.tensor_tensor(out=mhi[:], in0=i0[:], in1=bc(chi), op=OP.is_equal)
                    onem = wpool.tile([128, NSLOT], F32)
                    nc.scalar.activation(onem[:], w1[:], ACTF.Copy, bias=1.0, scale=-1.0)
                    wt0t = wpool.tile([128, NSLOT], F32)
                    wt1t = wpool.tile([128, NSLOT], F32)
                    wt0, wt1 = wt0t[:], wt1t[:]
                    ta = wpool.tile([128, NSLOT], F32)
                    nc.vector.tensor_tensor(out=ta[:], in0=onem[:], in1=mmid[:], op=OP.mult)
                    tb = wpool.tile([128, NSLOT], F32)
                    nc.vector.tensor_tensor(out=tb[:], in0=w1[:], in1=mlo[:], op=OP.mult)
                    nc.vector.tensor_tensor(out=wt0, in0=ta[:], in1=tb[:], op=OP.add)
                    nc.vector.tensor_tensor(out=wt0, in0=wt0, in1=w[:], op=OP.mult)
                    tc_ = wpool.tile([128, NSLOT], F32)
                    nc.vector.tensor_tensor(out=tc_[:], in0=w1[:], in1=mmid[:], op=OP.mult)
                    td = wpool.tile([128, NSLOT], F32)
                    nc.vector.tensor_tensor(out=td[:], in0=onem[:], in1=mhi[:], op=OP.mult)
                    nc.vector.tensor_tensor(out=wt1, in0=tc_[:], in1=td[:], op=OP.add)
                    nc.vector.tensor_tensor(out=wt1, in0=wt1, in1=w[:], op=OP.mult)

                    trw0 = ps_tr.tile([32, 128], F32)
                    nc.tensor.transpose(trw0[0:32, :], in_=wt0, identity=ident[:])
                    trw1 = ps_tr.tile([32, 128], F32)
                    nc.tensor.transpose(trw1[0:32, :], in_=wt1, identity=ident[:])
                    # replicate idx columns 8x in the free dim, then one
                    # transpose per 16-slot half directly yields all 128
                    # partitions (16-wrap x 8 Q7 groups) -- replaces 7
                    # serialized SBUF->SBUF DMAs (625ns HWDGE gen each)
                    rrep = wpool.tile([128, 2, 128], F32)
                    nc.vector.tensor_copy(out=rrep[:, 0, 0:16], in_=r[:, 0:16])
                    nc.vector.tensor_copy(out=rrep[:, 1, 0:16], in_=r[:, 16:32])
                    nc.vector.tensor_copy(out=rrep[:, :, 16:32], in_=rrep[:, :, 0:16])
                    nc.vector.tensor_copy(out=rrep[:, :, 32:64], in_=rrep[:, :, 0:32])
                    nc.vector.tensor_copy(out=rrep[:, :, 64:128], in_=rrep[:, :, 0:64])
                    trr = ps_tr.tile([128, 2, 128], F32, name="trr")
                    nc.tensor.transpose(trr[:, 0, :], in_=rrep[:, 0, :], identity=ident[:])
                    nc.tensor.transpose(trr[:, 1, :], in_=rrep[:, 1, :], identity=ident[:])

                    lhs0, lhs1, idxt = lhs0s[t], lhs1s[t], idxts[t]
                    lhsw1 = lhsw1s[t]
                    for i in range(4):
                        nc.vector.tensor_copy(out=lhs0[32 * i:32 * i + 32, i:128:4], in_=trw0[0:32, i:128:4])
                        if combine_mode == "mixed":
                            nc.vector.tensor_copy(out=lhsw1[32 * i:32 * i + 32, :, i:288:40],
                                                  in_=trw1[0:32, i:128:4])
                        else:
                            nc.vector.tensor_copy(out=lhs1[32 * i:32 * i + 32, i:128:4], in_=trw1[0:32, i:128:4])
                    nc.vector.tensor_copy(out=idxt[:, 0:2 * QT:2], in_=trr[:, 0, :])
                    nc.vector.tensor_copy(out=idxt[:, 1:2 * QT:2], in_=trr[:, 1, :])

                # ---------- loop 2: gather + combine + output ----------
                for t in range(NT):
                    lhs0, lhs1, idxt = lhs0s[t], lhs1s[t], idxts[t]
                    G = gpool.tile([128, NJ, 2 * D], F16, name="G")
                    jpc = NJ // calls_per_tile            # j-columns per call
                    npc = QT * NSLOT // calls_per_tile    # idxs per call
                    for gch in range(calls_per_tile):
                        nc.gpsimd.dma_gather(
                            out_ap=G[:, jpc * gch:jpc * (gch + 1), :],
                            in_ap=val2,
                            idxs_ap=idxt[:, (npc // 16) * gch:(npc // 16) * (gch + 1)],
                            num_idxs=npc,
                            num_idxs_reg=npc,
                            elem_size=2 * D,
                            elem_step=D,
                            single_packet=False,
                            queue_num=(t * calls_per_tile + gch) % gather_queues,
                        )

                    njs = NJ if combine else 1
                    if combine_mode == "mixed":
                        # dual-port combine: tap0 enters PE via the LDWEIGHTS
                        # port (G chunks stationary, rhs = 4-col weight blocks);
                        # tap1 via the rhs stream port (32-col zero-padded
                        # weight slabs stationary, G streamed, M=32 out block
                        # per query group b at 32-aligned psum partitions);
                        # then a transpose-accumulate folds the streamed half
                        # into the attnT psum.
                        lhsw1 = lhsw1s[t]
                        atT = ps_at.tile([128, 2, 128], F32)
                        atS = ps_at.tile([128, 2, 128], F32)
                        for u in range(8):
                            for b in range(4):
                                j = 8 * b + u
                                if j >= njs:
                                    continue
                                nc.tensor.matmul(atS[32 * b:32 * b + 32, :, :],
                                                 lhsT=lhsw1[:, b, 36 * u:36 * u + 32],
                                                 rhs=G[:, j, D:2 * D],
                                                 start=(u == 0), stop=(u == 7),
                                                 tile_position=(0, 32 * b),
                                                 skip_group_check=True)
                                for c in range(2):
                                    nc.tensor.matmul(atT[:, c, 4 * j:4 * j + 4],
                                                     lhsT=G[:, j, 128 * c:128 * c + 128],
                                                     rhs=lhs0[:, 4 * j:4 * j + 4],
                                                     start=(j == 0 and c == 0), stop=False,
                                                     skip_group_check=True)
                        atS_sb = wpool.tile([128, 2, 128], F32)
                        nc.vector.tensor_copy(out=atS_sb[:], in_=atS[:])
                        for c in range(2):
                            nc.tensor.matmul(atT[:, c, :], lhsT=atS_sb[:, c, :],
                                             rhs=ident[:], is_transpose=True,
                                             start=False, stop=True, skip_group_check=True)
                        attnT = wpool.tile([128, 2, 128], F16)
                        nc.vector.tensor_copy(out=attnT[:], in_=atT[:])
                    else:
                        at0 = ps_at.tile([128, 128], F32)
                        at1 = ps_at.tile([128, 128], F32)
                        atp = (at0, at1)
                        for j in range(njs):
                            for c in range(2):
                                nc.tensor.matmul(atp[c][:, 4 * j:4 * j + 4],
                                                 lhsT=G[:, j, 128 * c:128 * c + 128],
                                                 rhs=lhs0[:, 4 * j:4 * j + 4], start=True, stop=False)
                                nc.tensor.matmul(atp[c][:, 4 * j:4 * j + 4],
                                                 lhsT=G[:, j, D + 128 * c:D + 128 * c + 128],
                                                 rhs=lhs1[:, 4 * j:4 * j + 4], start=False, stop=True)
                        attnT = wpool.tile([128, 2, 128], F16)
                        nc.vector.tensor_copy(out=attnT[:, 0, :], in_=at0[:])
                        nc.vector.tensor_copy(out=attnT[:, 1, :], in_=at1[:])

                    op_ = ps_out.tile([128, D], F32)
                    nc.tensor.matmul(op_[:], lhsT=attnT[:, 0, :], rhs=woutf[:, 0, :], start=True, stop=False)
                    nc.tensor.matmul(op_[:], lhsT=attnT[:, 1, :], rhs=woutf[:, 1, :], start=False, stop=False)
                    nc.tensor.matmul(op_[:], lhsT=onesf[:1, :], rhs=boutf[:1, :], start=False, stop=True)
                    out_sb = wpool.tile([128, D], F32)
                    nc.vector.tensor_copy(out=out_sb[:], in_=op_[:])
                    nc.sync.dma_start(out=out_d[bass.ds(t * QT, QT), :], in_=out_sb[:])

    nc.compile()
    return nc


def make_in_maps(query, key, value, reference_points, W_off, b_off, W_attn, b_attn, W_out, b_out):
    query = np.asarray(query, dtype=np.float32)
    value = np.asarray(value, dtype=np.float32)
    reference_points = np.asarray(reference_points, dtype=np.float32)
    W_off = np.asarray(W_off, dtype=np.float32)
    b_off = np.asarray(b_off, dtype=np.float32)
    W_attn = np.asarray(W_attn, dtype=np.float32)
    b_attn = np.asarray(b_attn, dtype=np.float32)
    W_out = np.asarray(W_out, dtype=np.float32)
    b_out = np.asarray(b_out, dtype=np.float32)

    # fold grid_sample coordinate transform into the offset head (y columns only)
    wposf = (W_off[:, 1::2] * CPOS).astype(np.float32)           # [256, 32]
    bposf = (b_off[1::2] * CPOS).astype(np.float32)              # [32]

    def chunked(m, ncols):                                        # [256, n] -> [128, 2, n]
        return np.ascontiguousarray(m.reshape(2, 128, ncols).transpose(1, 0, 2))

    wpos_r = chunked(wposf, NSLOT)
    wattn_r = chunked(W_attn, NSLOT)
    wout_r = chunked(W_out, D)

    in_maps = []
    per_core = LQ // (N_CORES // B)                               # 1024
    for k in range(N_CORES):
        b = k // (N_CORES // B)
        q0 = (k % (N_CORES // B)) * per_core
        qs = query[b, q0:q0 + per_core, :]                        # [1024, 256]
        qT_r = np.ascontiguousarray(qs.T.reshape(2, 128, per_core).transpose(1, 0, 2))
        in_maps.append({
            "qT": qT_r,
            "refy": np.ascontiguousarray(reference_points[b, q0:q0 + per_core, 1][None, :]),
            "value": value[b].astype(np.float16),
            "wpos": wpos_r,
            "bpos": bposf[None, :],
            "wattn": wattn_r,
            "battn": b_attn[None, :].astype(np.float32),
            "wout": wout_r,
            "bout": b_out[None, :].astype(np.float32),
        })
    return in_maps


def kernel(**inputs) -> np.ndarray:
    from concourse.bass_utils import run_bass_kernel_spmd

    if "nc" not in _CACHE:
        _CACHE["nc"] = build_program()
    nc = _CACHE["nc"]
    in_maps = make_in_maps(**inputs)
    res = run_bass_kernel_spmd(nc, in_maps, list(range(N_CORES)), trace=False)
    shards = [res.results[k]["out"] for k in range(N_CORES)]
    out = np.empty((B, LQ, D), dtype=np.float32)
    per_core = LQ // (N_CORES // B)
    for k in range(N_CORES):
        b = k // (N_CORES // B)
        q0 = (k % (N_CORES // B)) * per_core
        out[b, q0:q0 + per_core, :] = shards[k]
    return out

